# revision 1
# baseline (speedup 1.0000x reference)
"""Trainium2 Bass kernel for the non-local-block module (nn_CNL_747324309589).

Sharding: data-parallel over batch — 16 batches across 8 NeuronCores, 2 per
core, no collectives.  Per batch (dims: HIGH=2048, LOW=512, N=H*W=1152):

    theta_xT[n,c] = sum_h xh[h,n]·thwT[h,c] + thb[c]      (x_h chunks = lhsT)
    phi_xT [n,d]  = sum_l xl[l,n]·phwT[l,d] + phb[d]      (phw,phb pre-scaled by 1/512)
    g_x    [d,n]  = sum_l gwT[l,d]·xl[l,n]  + gb[d]
    attT   [d,c]  = sum_n phi_xT[n,d]·theta_xT[n,c]       (= energy^T/512)
    y      [c,n]  = sum_d attT[d,c]·g_x[d,n]
    w_y    [o,n]  = sum_c wwT[c,o]·y[c,n]                 (BN scale pre-folded into ww)
    out    [o,n]  = w_y + bnt[o] + xh[o,n]                (shift + residual in one DVE op)

All matmuls run as float32r (full-rate PE at moving-dim >= 256) accumulating
fp32 in PSUM.  x_h stays resident in SBUF per batch (16x[128,1152]) serving
both as theta's lhsT chunks and the residual.  theta_wT and w_wT share one
4MB SBUF slot (disjoint phases, quarter-aligned byte ranges for progressive
turnover) so everything fits in the 28MiB SBUF.
"""

import numpy as np

import concourse.bass as bass
import concourse.bacc as bacc
import concourse.mybir as mybir
import concourse.tile as tile
from concourse.bass import ts

B, HIGH, LOW, H, W = 16, 2048, 512, 48, 24
N = H * W            # 1152
NCORES = 8
BPC = B // NCORES    # 2 batches per core
P = 128
KH = HIGH // P       # 16
KL = LOW // P        # 4
MN = N // P          # 9
NSPLIT = 3
NW = N // NSPLIT     # 384 (>=256 keeps float32r at full rate)
BN_EPS = 1e-5

F32 = mybir.dt.float32
F32R = mybir.dt.float32r
BF16 = mybir.dt.bfloat16
ADD = mybir.AluOpType.add
MULT = mybir.AluOpType.mult
AF = mybir.ActivationFunctionType


def _r(ap):
    return ap.bitcast(F32R)


def _build_module() -> bass.Bass:
    nc = bacc.Bacc()
    x_h = nc.dram_tensor("x_h", [BPC, HIGH, N], F32R, kind="ExternalInput")
    x_l = nc.dram_tensor("x_l", [BPC, LOW, N], F32R, kind="ExternalInput")
    thw = nc.dram_tensor("thw", [P, KH, LOW], F32R, kind="ExternalInput")
    phw = nc.dram_tensor("phw", [P, KL, LOW], F32R, kind="ExternalInput")
    gw = nc.dram_tensor("gw", [P, KL, LOW], F32R, kind="ExternalInput")
    # ww laid out [P, o-quarter, KL, 512] so each quarter is byte-aligned with
    # a thw k-quarter in the shared SBUF slot (progressive slot turnover)
    ww = nc.dram_tensor("ww", [P, 4, KL, HIGH // 4], F32R, kind="ExternalInput")
    thpb = nc.dram_tensor("thpb", [1, 2 * LOW], BF16, kind="ExternalInput")
    gbnt = nc.dram_tensor("gbnt", [P, KL + KH], F32, kind="ExternalInput")
    out = nc.dram_tensor("out", [BPC, HIGH, N], F32, kind="ExternalOutput")

    with tile.TileContext(nc) as tc:
        with (
            tc.tile_pool(name="consts", bufs=1) as cpool,
            tc.tile_pool(name="bigw", bufs=1) as wpool,
            tc.tile_pool(name="xh", bufs=KH) as xhpool,
            tc.tile_pool(name="xl", bufs=1) as xlpool,
            tc.tile_pool(name="mid", bufs=1) as midpool,
            tc.tile_pool(name="stg", bufs=9) as stgpool,
            tc.tile_pool(name="psum", bufs=8, space="PSUM") as pspool,
        ):
            # first batch's x_l and the phi weights go first so phase A2 can
            # start as early as possible; constants are packed into few DMAs
            # because serialized DMA-issue time paces the prologue
            xl0_sb = xlpool.tile([P, KL, N], F32R, tag="xl")
            nc.sync.dma_start(xl0_sb[:], x_l[0].rearrange("(ko p) n -> p ko n", p=P))
            phw_sb = cpool.tile([P, KL, LOW], F32R, tag="phw")
            nc.sync.dma_start(phw_sb[:], phw[:])
            gw_sb = cpool.tile([P, KL, LOW], F32R, tag="gw")
            nc.sync.dma_start(gw_sb[:], gw[:])
            thpb_sb = cpool.tile([P, 2 * LOW], BF16, tag="thpb")
            nc.sync.dma_start(thpb_sb[:], thpb[:].to_broadcast((P, 2 * LOW)))
            thb_sb = thpb_sb[:, :LOW]
            phb_sb = thpb_sb[:, LOW:]
            gbnt_sb = cpool.tile([P, KL + KH], F32, tag="gbnt")
            nc.sync.dma_start(gbnt_sb[:], gbnt[:])
            gb_sb = gbnt_sb[:, :KL]
            bnt_sb = gbnt_sb[:, KL:]

            for b in range(BPC):
                if b == 0:
                    xl_sb = xl0_sb
                else:
                    xl_sb = xlpool.tile([P, KL, N], F32R, tag="xl")
                    nc.sync.dma_start(
                        xl_sb[:], x_l[b].rearrange("(ko p) n -> p ko n", p=P)
                    )
                # interleave theta-weight quarters with x_h chunks so the
                # theta k-loop can start as soon as the first pieces land
                thw_sb = wpool.tile([P, KH, LOW], F32R, tag="bigw")
                xh_t = []
                for q in range(4):
                    nc.sync.dma_start(
                        thw_sb[:, ts(q, KH // 4), :], thw[:, ts(q, KH // 4), :]
                    )
                    for k in range(q * 4, q * 4 + 4):
                        t_ = xhpool.tile([P, N], F32R, tag="xh")
                        nc.sync.dma_start(t_[:], x_h[b, ts(k, P), :])
                        xh_t.append(t_)

                # phi_xT [n, d] (phase A2)
                ph_sb = midpool.tile([P, MN, LOW], F32R, tag="ph")
                for m in range(MN):
                    ps = pspool.tile([P, 512], F32, tag="ps")
                    for k in range(KL):
                        nc.tensor.matmul(
                            ps[:],
                            _r(xl_sb[:, k, ts(m, P)]),
                            _r(phw_sb[:, k, :]),
                            start=(k == 0),
                            stop=(k == KL - 1),
                        )
                    nc.vector.tensor_tensor(ph_sb[:, m, :], ps[:], phb_sb[:], ADD)

                # g_x [d, n] (phase A3)
                g_sb = midpool.tile([P, KL, N], F32R, tag="g")
                for md in range(KL):
                    for nn in range(NSPLIT):
                        ps = pspool.tile([P, 512], F32, tag="ps")
                        for k in range(KL):
                            nc.tensor.matmul(
                                ps[:, :NW],
                                _r(gw_sb[:, k, ts(md, P)]),
                                _r(xl_sb[:, k, ts(nn, NW)]),
                                start=(k == 0),
                                stop=(k == KL - 1),
                            )
                        nc.scalar.activation(
                            g_sb[:, md, ts(nn, NW)],
                            ps[:, :NW],
                            AF.Identity,
                            bias=gb_sb[:, md : md + 1],
                        )

                # theta_xT [n, c] (phase A1)
                th_sb = midpool.tile([P, MN, LOW], F32R, tag="th")
                for m in range(MN):
                    ps = pspool.tile([P, 512], F32, tag="ps")
                    for k in range(KH):
                        nc.tensor.matmul(
                            ps[:],
                            _r(xh_t[k][:, ts(m, P)]),
                            _r(thw_sb[:, k, :]),
                            start=(k == 0),
                            stop=(k == KH - 1),
                        )
                    nc.vector.tensor_tensor(th_sb[:, m, :], ps[:], thb_sb[:], ADD)

                # attT [d, c] = energy^T/512 (phase B1); parks in the xl slot
                # (xl is dead after A3, reloaded for b+1 only after B2 reads)
                att_sb = xlpool.tile([P, KL, LOW], F32R, tag="xl")
                for md in range(KL):
                    ps = pspool.tile([P, 512], F32, tag="ps")
                    for k in range(MN):
                        nc.tensor.matmul(
                            ps[:],
                            _r(ph_sb[:, k, ts(md, P)]),
                            _r(th_sb[:, k, :]),
                            start=(k == 0),
                            stop=(k == MN - 1),
                        )
                    nc.scalar.activation(att_sb[:, md, :], ps[:], AF.Copy)

                # y [c, n] (phase B2)
                # y shares the theta_xT slot: th is dead after B1, same byte size
                y_sb = midpool.tile([P, KL, N], F32R, tag="th")
                for mc in range(KL):
                    for nn in range(NSPLIT):
                        ps = pspool.tile([P, 512], F32, tag="ps")
                        for k in range(KL):
                            nc.tensor.matmul(
                                ps[:, :NW],
                                _r(att_sb[:, k, ts(mc, P)]),
                                _r(g_sb[:, k, ts(nn, NW)]),
                                start=(k == 0),
                                stop=(k == KL - 1),
                            )
                        nc.scalar.activation(y_sb[:, mc, ts(nn, NW)], ps[:, :NW], AF.Copy)

                # w_y + BN + residual (phase C)
                ww_sb = wpool.tile([P, 4, KL, HIGH // 4], F32R, tag="bigw")
                for q in range(4):
                    nc.sync.dma_start(ww_sb[:, q], ww[:, q])
                for mo in range(KH):
                    xt = xh_t[mo]
                    for nn in range(NSPLIT):
                        ps = pspool.tile([P, 512], F32, tag="ps")
                        for k in range(KL):
                            nc.tensor.matmul(
                                ps[:, :NW],
                                _r(ww_sb[:, mo // 4, k, ts(mo % 4, P)]),
                                _r(y_sb[:, k, ts(nn, NW)]),
                                start=(k == 0),
                                stop=(k == KL - 1),
                            )
                        stg = stgpool.tile([P, NW], F32, tag="stg")
                        nc.vector.scalar_tensor_tensor(
                            stg[:],
                            ps[:, :NW],
                            bnt_sb[:, mo : mo + 1],
                            xt[:, ts(nn, NW)].bitcast(F32),
                            ADD,
                            ADD,
                        )
                        nc.sync.dma_start(out[b, ts(mo, P), ts(nn, NW)], stg[:])
    nc.compile()
    return nc


_CACHE: dict = {}


def _get_module() -> bass.Bass:
    if "nc" not in _CACHE:
        _CACHE["nc"] = _build_module()
    return _CACHE["nc"]


def _prep_maps(inputs: dict) -> list[dict]:
    f = lambda a: np.ascontiguousarray(np.asarray(a, dtype=np.float32))
    x_h = f(inputs["x_h"]).reshape(B, HIGH, N)
    x_l = f(inputs["x_l"]).reshape(B, LOW, N)
    theta_w = f(inputs["theta_w"])
    phi_w = f(inputs["phi_w"])
    g_w = f(inputs["g_w"])
    w_w = f(inputs["w_w"])

    thw_h = np.ascontiguousarray(theta_w.T.reshape(KH, P, LOW).transpose(1, 0, 2))
    phw_h = np.ascontiguousarray((phi_w.T / np.float32(LOW)).reshape(KL, P, LOW).transpose(1, 0, 2))
    gw_h = np.ascontiguousarray(g_w.T.reshape(KL, P, LOW).transpose(1, 0, 2))
    s = f(inputs["bn_gamma"]) / np.sqrt(f(inputs["bn_var"]) + np.float32(BN_EPS))
    # BN scale folded into the w conv weights; only the shift remains on-device
    ww_h = np.ascontiguousarray(
        (w_w * s[:, None])
        .astype(np.float32)
        .T.reshape(KL, P, 4, HIGH // 4)
        .transpose(1, 2, 0, 3)
    )

    import ml_dtypes
    thpb_h = np.concatenate(
        [f(inputs["theta_b"]), f(inputs["phi_b"]) / np.float32(LOW)]
    ).reshape(1, 2 * LOW).astype(ml_dtypes.bfloat16)
    gb_h = np.ascontiguousarray(f(inputs["g_b"]).reshape(KL, P).T)
    t = (f(inputs["w_b"]) - f(inputs["bn_mean"])) * s + f(inputs["bn_beta"])
    bnt_h = np.ascontiguousarray(t.astype(np.float32).reshape(KH, P).T)
    gbnt_h = np.ascontiguousarray(np.concatenate([gb_h, bnt_h], axis=1))

    shared = dict(
        thw=thw_h, phw=phw_h, gw=gw_h, ww=ww_h, thpb=thpb_h, gbnt=gbnt_h,
    )
    maps = []
    for c in range(NCORES):
        m = dict(shared)
        m["x_h"] = np.ascontiguousarray(x_h[c * BPC : (c + 1) * BPC])
        m["x_l"] = np.ascontiguousarray(x_l[c * BPC : (c + 1) * BPC])
        maps.append(m)
    return maps


def _run(inputs: dict, **kwargs):
    from concourse.bass_utils import run_bass_kernel_spmd

    nc = _get_module()
    in_maps = _prep_maps(inputs)
    res = run_bass_kernel_spmd(nc, in_maps, core_ids=list(range(NCORES)), **kwargs)
    parts = [r["out"] for r in res.results]
    full = np.concatenate(parts, axis=0).reshape(B, HIGH, H, W)
    return full, res


def kernel(**inputs) -> np.ndarray:
    full, _ = _run(inputs)
    return full



# revision 6
# speedup vs baseline: 1.1305x; 1.1305x over previous
"""Trainium2 Bass kernel for the non-local-block module (nn_CNL_747324309589).

Sharding: data-parallel over batch — 16 batches across 8 NeuronCores, 2 per
core, no collectives.  Per batch (dims: HIGH=2048, LOW=512, N=H*W=1152):

    theta_xT[n,c] = sum_h xh[h,n]·thwT[h,c] + thb[c]       (A2)
    phi_xT [n,d]  = sum_l xl[l,n]·phwT[l,d] + phb[d]       (A1, phw,phb /512)
    attT   [d,c]  = sum_n phi_xT[n,d]·theta_xT[n,c]        (B1, = energy^T/512)
    stT    [l,c]  = sum_d gw[d,l]·attT[d,c]                (B2, = (att@g_w)^T)
    agb    [c]    = sum_d attT[d,c]·gb[d]                  (AGB, tiny)
    y      [c,n]  = sum_l stT[l,c]·xl[l,n] + agb[c]        (B3)
    w_y    [o,n]  = sum_c wwT[c,o]·y[c,n]                  (C, BN scale in ww)
    out    [o,n]  = w_y + bnt[o] + xh[o,n]                 (stst + residual)

The B2/B3 re-association (y = (att·[g_w])·x_l + (att·g_b)) eliminates the
separate g_x = g_w·x_l + g_b phase: 8192+16 rows replace 18432 per batch.

Everything moves as bfloat16 (PE rate for bf16 == float32r at 1.0 cycle/row,
but DMA bytes and SBUF halve: total DMA ~76us vs PE ~176us so the single
DMA resource never paces the kernel).  PSUM accumulates fp32.  Measured
end-to-end rel err ~6e-3 (gate 2e-2).

Schedule: batch 0's theta streams x_h m-column-slices (host pre-tiled
[p, m, kh, 128]) so the first theta tile starts right after A1; batch 1's
x_h is fully resident by its theta (normal [kh, n] chunks).  Cross-batch
filler tiles (A1 of b1 / held-back C tiles of b0) cover every phase-boundary
PSUM-evacuation latency so the PE never idles mid-kernel.
"""

import numpy as np

import concourse.bass as bass
import concourse.bacc as bacc
import concourse.mybir as mybir
import concourse.tile as tile
from concourse.bass import ts

B, HIGH, LOW, H, W = 16, 2048, 512, 48, 24
N = H * W            # 1152
NCORES = 8
BPC = B // NCORES    # 2 batches per core
P = 128
KH = HIGH // P       # 16
KL = LOW // P        # 4
MN = N // P          # 9
NSPLIT = 3
NW = N // NSPLIT     # 384
BN_EPS = 1e-5

F32 = mybir.dt.float32
BF16 = mybir.dt.bfloat16
ADD = mybir.AluOpType.add
AF = mybir.ActivationFunctionType


def _build_module() -> bass.Bass:
    nc = bacc.Bacc()
    # batch-0 x_h pre-tiled by n-column block m: x_h0[p, m, kh, c] = xh[kh*128+p, m*128+c]
    x_h0 = nc.dram_tensor("x_h0", [P, MN, KH, P], BF16, kind="ExternalInput")
    # batch-1 x_h in plain channel-major layout
    x_h1 = nc.dram_tensor("x_h1", [HIGH, N], BF16, kind="ExternalInput")
    x_l = nc.dram_tensor("x_l", [BPC, P, KL, N], BF16, kind="ExternalInput")
    thw = nc.dram_tensor("thw", [P, KH, LOW], BF16, kind="ExternalInput")
    phw = nc.dram_tensor("phw", [P, KL, LOW], BF16, kind="ExternalInput")
    gw = nc.dram_tensor("gw", [P, KL, LOW], BF16, kind="ExternalInput")
    ww = nc.dram_tensor("ww", [P, KL, HIGH], BF16, kind="ExternalInput")
    thpb = nc.dram_tensor("thpb", [1, 2 * LOW], BF16, kind="ExternalInput")
    gbc = nc.dram_tensor("gbc", [P, KL], BF16, kind="ExternalInput")
    bnt = nc.dram_tensor("bnt", [P, KH], F32, kind="ExternalInput")
    out = nc.dram_tensor("out", [BPC, HIGH, N], BF16, kind="ExternalOutput")

    with tile.TileContext(nc) as tc:
        with (
            tc.tile_pool(name="consts", bufs=1) as cpool,
            tc.tile_pool(name="xh0", bufs=MN) as xh0pool,
            tc.tile_pool(name="xh1", bufs=KH) as xh1pool,
            tc.tile_pool(name="xl", bufs=2) as xlpool,
            tc.tile_pool(name="ph", bufs=2) as phpool,
            tc.tile_pool(name="mid", bufs=1) as midpool,
            tc.tile_pool(name="stg", bufs=4) as stgpool,
            tc.tile_pool(name="psum", bufs=8, space="PSUM") as pspool,
        ):
            # ---------------- DMA stream (single global DMA resource; this
            # order is the delivery schedule the PE schedule below relies on)
            phw_sb = cpool.tile([P, KL, LOW], BF16, tag="phw")
            nc.sync.dma_start(phw_sb[:], phw[:])
            thpb_sb = cpool.tile([P, 2 * LOW], BF16, tag="thpb")
            nc.sync.dma_start(thpb_sb[:], thpb[:].to_broadcast((P, 2 * LOW)))
            thb_sb = thpb_sb[:, :LOW]
            phb_sb = thpb_sb[:, LOW:]
            xl_sb = [None, None]
            xl_sb[0] = xlpool.tile([P, KL, N], BF16, tag="xl", name="xl0_sb")
            nc.sync.dma_start(xl_sb[0][:, :, 0:NW], x_l[0][:, :, 0:NW])
            nc.sync.dma_start(xl_sb[0][:, :, NW:N], x_l[0][:, :, NW:N])
            thw_sb = cpool.tile([P, KH, LOW], BF16, tag="thw")
            xh0_t = []
            nc.sync.dma_start(thw_sb[:, ts(0, 4), :], thw[:, ts(0, 4), :])
            t_ = xh0pool.tile([P, KH, P], BF16, tag="xh0")
            nc.sync.dma_start(t_[:], x_h0[:, 0])
            xh0_t.append(t_)
            nc.sync.dma_start(thw_sb[:, ts(1, 4), :], thw[:, ts(1, 4), :])
            nc.sync.dma_start(thw_sb[:, ts(2, 4), :], thw[:, ts(2, 4), :])
            nc.sync.dma_start(thw_sb[:, ts(3, 4), :], thw[:, ts(3, 4), :])
            for m in range(1, MN):
                t_ = xh0pool.tile([P, KH, P], BF16, tag="xh0")
                nc.sync.dma_start(t_[:], x_h0[:, m])
                xh0_t.append(t_)
            gw_sb = cpool.tile([P, KL, LOW], BF16, tag="gw")
            nc.sync.dma_start(gw_sb[:], gw[:])
            gbc_sb = cpool.tile([P, KL], BF16, tag="gbc")
            nc.sync.dma_start(gbc_sb[:], gbc[:])
            bnt_sb = cpool.tile([P, KH], F32, tag="bnt")
            nc.sync.dma_start(bnt_sb[:], bnt[:])
            xl_sb[1] = xlpool.tile([P, KL, N], BF16, tag="xl", name="xl1_sb")
            nc.sync.dma_start(xl_sb[1][:], x_l[1][:])
            ww_sb = cpool.tile([P, KL, HIGH], BF16, tag="ww")
            nc.sync.dma_start(ww_sb[:], ww[:])
            xh1_t = []
            for k in range(KH):
                t_ = xh1pool.tile([P, N], BF16, tag="xh1")
                nc.sync.dma_start(t_[:], x_h1[ts(k, P), :])
                xh1_t.append(t_)

            # ---------------- persistent intermediates
            ph_sb = [
                phpool.tile([P, MN, LOW], BF16, tag="ph", name=f"ph{i}_sb")
                for i in range(2)
            ]
            th_sb = midpool.tile([P, MN, LOW], BF16, tag="th")
            att_sb = midpool.tile([P, KL, LOW], BF16, tag="att")
            st_sb = midpool.tile([P, KL, LOW], BF16, tag="st")
            agb_sb = midpool.tile([P, KL], F32, tag="agb")
            y_sb = midpool.tile([P, KL, N], BF16, tag="y")

            # ---------------- phase helpers (emission order == schedule)
            def a1_tile(b, m):
                # phi_xT tile: out [n-part m, d 512]
                ps = pspool.tile([P, 512], F32, tag="ps")
                for k in range(KL):
                    nc.tensor.matmul(
                        ps[:], xl_sb[b][:, k, ts(m, P)], phw_sb[:, k, :],
                        start=(k == 0), stop=(k == KL - 1),
                    )
                nc.vector.tensor_tensor(ph_sb[b][:, m, :], ps[:], phb_sb[:], ADD)

            def a2_tile_b0(m):
                # theta_xT tile from m-sliced x_h0: out [n-part m, c 512]
                ps = pspool.tile([P, 512], F32, tag="ps")
                for k in range(KH):
                    nc.tensor.matmul(
                        ps[:], xh0_t[m][:, k, :], thw_sb[:, k, :],
                        start=(k == 0), stop=(k == KH - 1),
                    )
                nc.vector.tensor_tensor(th_sb[:, m, :], ps[:], thb_sb[:], ADD)

            def a2_tile_b1(m):
                ps = pspool.tile([P, 512], F32, tag="ps")
                for k in range(KH):
                    nc.tensor.matmul(
                        ps[:], xh1_t[k][:, ts(m, P)], thw_sb[:, k, :],
                        start=(k == 0), stop=(k == KH - 1),
                    )
                nc.vector.tensor_tensor(th_sb[:, m, :], ps[:], thb_sb[:], ADD)

            def b1_tile(b, md):
                # attT tile: out [d-part md, c 512]
                ps = pspool.tile([P, 512], F32, tag="ps")
                for k in range(MN):
                    nc.tensor.matmul(
                        ps[:], ph_sb[b][:, k, ts(md, P)], th_sb[:, k, :],
                        start=(k == 0), stop=(k == MN - 1),
                    )
                nc.scalar.activation(att_sb[:, md, :], ps[:], AF.Copy)

            def b2_tile(ml):
                # stT tile: out [l-part ml, c 512]
                ps = pspool.tile([P, 512], F32, tag="ps")
                for k in range(KL):
                    nc.tensor.matmul(
                        ps[:], gw_sb[:, k, ts(ml, P)], att_sb[:, k, :],
                        start=(k == 0), stop=(k == KL - 1),
                    )
                nc.scalar.activation(st_sb[:, ml, :], ps[:], AF.Copy)

            def agb_tiles():
                # agb[c] = sum_d attT[d,c]·gb[d]: out [c-part mc, 1]
                for mc in range(KL):
                    ps = pspool.tile([P, 512], F32, tag="ps")
                    for k in range(KL):
                        nc.tensor.matmul(
                            ps[:, 0:1], att_sb[:, k, ts(mc, P)], gbc_sb[:, k : k + 1],
                            start=(k == 0), stop=(k == KL - 1),
                        )
                    nc.scalar.activation(agb_sb[:, mc : mc + 1], ps[:, 0:1], AF.Copy)

            def b3_tile(b, nn, mc):
                # y tile: out [c-part mc, n-slice nn]
                ps = pspool.tile([P, 512], F32, tag="ps")
                for k in range(KL):
                    nc.tensor.matmul(
                        ps[:, :NW], st_sb[:, k, ts(mc, P)], xl_sb[b][:, k, ts(nn, NW)],
                        start=(k == 0), stop=(k == KL - 1),
                    )
                nc.scalar.activation(
                    y_sb[:, mc, ts(nn, NW)], ps[:, :NW], AF.Identity,
                    bias=agb_sb[:, mc : mc + 1],
                )

            def c_tile(b, mo, nn):
                # w_y tile + BN shift + residual, then store
                ps = pspool.tile([P, 512], F32, tag="ps")
                for k in range(KL):
                    nc.tensor.matmul(
                        ps[:, :NW], ww_sb[:, k, ts(mo, P)], y_sb[:, k, ts(nn, NW)],
                        start=(k == 0), stop=(k == KL - 1),
                    )
                stg = stgpool.tile([P, NW], BF16, tag="stg")
                if b == 0:
                    for j in range(NSPLIT):
                        nc.vector.scalar_tensor_tensor(
                            stg[:, ts(j, P)], ps[:, ts(j, P)], bnt_sb[:, mo : mo + 1],
                            xh0_t[nn * NSPLIT + j][:, mo, :], ADD, ADD,
                        )
                else:
                    nc.vector.scalar_tensor_tensor(
                        stg[:], ps[:, :NW], bnt_sb[:, mo : mo + 1],
                        xh1_t[mo][:, ts(nn, NW)], ADD, ADD,
                    )
                nc.sync.dma_start(out[b, ts(mo, P), ts(nn, NW)], stg[:])

            # ---------------- the schedule
            for m in range(MN):
                a1_tile(0, m)                       # A1(b0)
            for m in range(MN):
                a2_tile_b0(m)                       # A2(b0), streams x_h0
            a1_tile(1, 0)                           # filler: th evac latency
            a1_tile(1, 1)
            for md in range(KL):
                b1_tile(0, md)                      # B1(b0)
            a1_tile(1, 2)                           # filler: att evac
            for ml in range(KL):
                b2_tile(ml)                         # B2(b0)
            agb_tiles()
            a1_tile(1, 3)                           # filler: st evac
            for nn in range(NSPLIT):
                for mc in range(KL):
                    b3_tile(0, nn, mc)              # B3(b0), nn-outer
            for nn in range(2):
                for mo in range(KH):
                    c_tile(0, mo, nn)               # C(b0) nn0+nn1
            for m in range(4, MN):
                a1_tile(1, m)                       # A1(b1) rest
            for m in range(MN):
                a2_tile_b1(m)                       # A2(b1), x_h1 resident
            for mo in range(0, 4):
                c_tile(0, mo, 2)                    # filler: th(b1) evac
            for md in range(KL):
                b1_tile(1, md)                      # B1(b1)
            for mo in range(4, 8):
                c_tile(0, mo, 2)                    # filler: att evac
            for ml in range(KL):
                b2_tile(ml)                         # B2(b1)
            agb_tiles()
            for mo in range(8, KH):
                c_tile(0, mo, 2)                    # filler: st evac + C(b0) tail
            for nn in range(NSPLIT):
                for mc in range(KL):
                    b3_tile(1, nn, mc)              # B3(b1)
            for nn in range(NSPLIT):
                for mo in range(KH):
                    c_tile(1, mo, nn)               # C(b1)
    nc.compile()
    return nc


_CACHE: dict = {}


def _get_module() -> bass.Bass:
    if "nc" not in _CACHE:
        _CACHE["nc"] = _build_module()
    return _CACHE["nc"]


def _prep_maps(inputs: dict) -> list[dict]:
    import ml_dtypes

    bf = ml_dtypes.bfloat16
    f = lambda a: np.asarray(a, dtype=np.float32)
    x_h = f(inputs["x_h"]).reshape(B, HIGH, N)
    x_l = f(inputs["x_l"]).reshape(B, LOW, N)
    theta_w = f(inputs["theta_w"])
    phi_w = f(inputs["phi_w"])
    g_w = f(inputs["g_w"])
    w_w = f(inputs["w_w"])

    thw_h = np.ascontiguousarray(
        theta_w.T.reshape(KH, P, LOW).transpose(1, 0, 2).astype(bf))
    phw_h = np.ascontiguousarray(
        (phi_w.T / np.float32(LOW)).reshape(KL, P, LOW).transpose(1, 0, 2).astype(bf))
    # gw in [d-part, l] layout (NOT transposed): gw_h[p,k,l] = g_w[k*128+p, l]
    gw_h = np.ascontiguousarray(g_w.reshape(KL, P, LOW).transpose(1, 0, 2).astype(bf))
    s = f(inputs["bn_gamma"]) / np.sqrt(f(inputs["bn_var"]) + np.float32(BN_EPS))
    # BN scale folded into the w conv weights; only the shift remains on-device
    ww_h = np.ascontiguousarray(
        (w_w * s[:, None]).T.reshape(KL, P, HIGH).transpose(1, 0, 2).astype(bf))

    thpb_h = np.concatenate(
        [f(inputs["theta_b"]), f(inputs["phi_b"]) / np.float32(LOW)]
    ).reshape(1, 2 * LOW).astype(bf)
    gbc_h = np.ascontiguousarray(f(inputs["g_b"]).reshape(KL, P).T.astype(bf))
    t = (f(inputs["w_b"]) - f(inputs["bn_mean"])) * s + f(inputs["bn_beta"])
    bnt_h = np.ascontiguousarray(t.astype(np.float32).reshape(KH, P).T)

    x_h_bf = x_h.astype(bf)
    x_l_bf = x_l.astype(bf)
    shared = dict(
        thw=thw_h, phw=phw_h, gw=gw_h, ww=ww_h, thpb=thpb_h, gbc=gbc_h, bnt=bnt_h,
    )
    maps = []
    for c in range(NCORES):
        m = dict(shared)
        b0 = x_h_bf[c * BPC]
        m["x_h0"] = np.ascontiguousarray(
            b0.reshape(KH, P, MN, P).transpose(1, 2, 0, 3))
        m["x_h1"] = np.ascontiguousarray(x_h_bf[c * BPC + 1])
        m["x_l"] = np.ascontiguousarray(
            x_l_bf[c * BPC : (c + 1) * BPC].reshape(BPC, KL, P, N).transpose(0, 2, 1, 3))
        maps.append(m)
    return maps


def _run(inputs: dict, **kwargs):
    from concourse.bass_utils import run_bass_kernel_spmd

    nc = _get_module()
    in_maps = _prep_maps(inputs)
    res = run_bass_kernel_spmd(nc, in_maps, core_ids=list(range(NCORES)), **kwargs)
    parts = [np.asarray(r["out"], dtype=np.float32) for r in res.results]
    full = np.concatenate(parts, axis=0).reshape(B, HIGH, H, W)
    return full, res


def kernel(**inputs) -> np.ndarray:
    full, _ = _run(inputs)
    return full


# revision 11
# speedup vs baseline: 1.1451x; 1.0130x over previous
"""Trainium2 Bass kernel for the non-local-block module (nn_CNL_747324309589).

Sharding: data-parallel over batch — 16 batches across 8 NeuronCores, 2 per
core, no collectives.  Per batch (dims: HIGH=2048, LOW=512, N=H*W=1152):

    theta_xT[n,c] = sum_h xh[h,n]·thwT[h,c] + thb[c]       (A2)
    phi_xT [n,d]  = sum_l xl[l,n]·phwT[l,d] + phb[d]       (A1, phw,phb /512)
    attT   [d,c]  = sum_n phi_xT[n,d]·theta_xT[n,c]        (B1, = energy^T/512)
    stT    [l,c]  = sum_d gw[d,l]·attT[d,c]                (B2, = (att@g_w)^T)
    agb    [c]    = sum_d attT[d,c]·gb[d]                  (AGB, tiny)
    y      [c,n]  = sum_l stT[l,c]·xl[l,n] + agb[c]        (B3)
    w_y    [o,n]  = sum_c wwT[c,o]·y[c,n]                  (C, BN scale in ww)
    out    [o,n]  = w_y + bnt[o] + xh[o,n]                 (stst + residual)

The B2/B3 re-association (y = (att·[g_w])·x_l + (att·g_b)) eliminates the
separate g_x = g_w·x_l + g_b phase: 8192+16 rows replace 18432 per batch.

Everything moves as bfloat16 (PE rate for bf16 == float32r at 1.0 cycle/row,
but DMA bytes and SBUF halve: total DMA ~76us vs PE ~176us so the single
DMA resource never paces the kernel).  PSUM accumulates fp32.  Measured
end-to-end rel err ~6e-3 (gate 2e-2).

Schedule: batch 0's theta streams x_h m-column-slices (host pre-tiled
[p, m, kh, 128]) so the first theta tile starts right after A1; batch 1's
x_h is fully resident by its theta (normal [kh, n] chunks).  Cross-batch
filler tiles (A1 of b1 / held-back C tiles of b0) cover every phase-boundary
PSUM-evacuation latency so the PE never idles mid-kernel.
"""

import numpy as np

import concourse.bass as bass
import concourse.bacc as bacc
import concourse.mybir as mybir
import concourse.tile as tile
from concourse.bass import ts

B, HIGH, LOW, H, W = 16, 2048, 512, 48, 24
N = H * W            # 1152
NCORES = 8
BPC = B // NCORES    # 2 batches per core
P = 128
KH = HIGH // P       # 16
KL = LOW // P        # 4
MN = N // P          # 9
NSPLIT = 3
NW = N // NSPLIT     # 384
BN_EPS = 1e-5

F32 = mybir.dt.float32
BF16 = mybir.dt.bfloat16
ADD = mybir.AluOpType.add
AF = mybir.ActivationFunctionType


def _build_module() -> bass.Bass:
    nc = bacc.Bacc()
    # batch-0 x_h pre-tiled by n-column block m: x_h0[p, m, kh, c] = xh[kh*128+p, m*128+c]
    x_h0 = nc.dram_tensor("x_h0", [P, MN, KH, P], BF16, kind="ExternalInput")
    # batch-1 x_h in plain channel-major layout
    x_h1 = nc.dram_tensor("x_h1", [HIGH, N], BF16, kind="ExternalInput")
    x_l = nc.dram_tensor("x_l", [BPC, P, KL, N], BF16, kind="ExternalInput")
    thw = nc.dram_tensor("thw", [P, KH, LOW], BF16, kind="ExternalInput")
    phw = nc.dram_tensor("phw", [P, KL, LOW], BF16, kind="ExternalInput")
    gw = nc.dram_tensor("gw", [P, KL, LOW], BF16, kind="ExternalInput")
    ww = nc.dram_tensor("ww", [P, KL, HIGH], BF16, kind="ExternalInput")
    thpb = nc.dram_tensor("thpb", [1, 2 * LOW], BF16, kind="ExternalInput")
    gbc = nc.dram_tensor("gbc", [P, KL], BF16, kind="ExternalInput")
    bnt = nc.dram_tensor("bnt", [P, KH], F32, kind="ExternalInput")
    out = nc.dram_tensor("out", [BPC, HIGH, N], BF16, kind="ExternalOutput")

    with tile.TileContext(nc) as tc:
        with (
            tc.tile_pool(name="consts", bufs=1) as cpool,
            tc.tile_pool(name="xh0", bufs=MN) as xh0pool,
            tc.tile_pool(name="xh1", bufs=KH) as xh1pool,
            tc.tile_pool(name="xl", bufs=2) as xlpool,
            tc.tile_pool(name="ph", bufs=2) as phpool,
            tc.tile_pool(name="mid", bufs=1) as midpool,
            tc.tile_pool(name="stg", bufs=4) as stgpool,
            tc.tile_pool(name="psum", bufs=8, space="PSUM") as pspool,
        ):
            # ---------------- PE warmup: dummy matmuls on a memset tile keep
            # the PE busy through the DMA prologue so the p-state ramp (half
            # clock for the first 3us of a busy stretch) is spent on idle
            # time instead of real matmuls.
            wt_sb = cpool.tile([P, 64], BF16, tag="wt")
            nc.vector.memset(wt_sb[:], 0)
            wps = pspool.tile([P, 512], F32, tag="ps", name="wps")
            for _ in range(80):
                nc.tensor.matmul(wps[0:64, 0:64], wt_sb[:], wt_sb[:],
                                 start=True, stop=True)

            # ---------------- DMA stream (single global DMA resource; this
            # order is the delivery schedule the PE schedule below relies on)
            phw_sb = cpool.tile([P, KL, LOW], BF16, tag="phw")
            nc.sync.dma_start(phw_sb[:], phw[:])
            xl_sb = [None, None]
            xl_sb[0] = xlpool.tile([P, KL, N], BF16, tag="xl", name="xl0_sb")
            nc.sync.dma_start(xl_sb[0][:, :, 0:NW], x_l[0][:, :, 0:NW])
            nc.sync.dma_start(xl_sb[0][:, :, NW:N], x_l[0][:, :, NW:N])
            thw_sb = cpool.tile([P, KH, LOW], BF16, tag="thw")
            xh0_t = []
            nc.sync.dma_start(thw_sb[:, ts(0, 4), :], thw[:, ts(0, 4), :])
            t_ = xh0pool.tile([P, KH, P], BF16, tag="xh0")
            nc.sync.dma_start(t_[:], x_h0[:, 0])
            xh0_t.append(t_)
            thpb_sb = cpool.tile([P, 2 * LOW], BF16, tag="thpb")
            nc.sync.dma_start(thpb_sb[:], thpb[:].to_broadcast((P, 2 * LOW)))
            thb_sb = thpb_sb[:, :LOW]
            phb_sb = thpb_sb[:, LOW:]
            nc.sync.dma_start(thw_sb[:, ts(1, 4), :], thw[:, ts(1, 4), :])
            for m in range(1, 4):
                t_ = xh0pool.tile([P, KH, P], BF16, tag="xh0")
                nc.sync.dma_start(t_[:], x_h0[:, m])
                xh0_t.append(t_)
            nc.sync.dma_start(thw_sb[:, ts(2, 4), :], thw[:, ts(2, 4), :])
            nc.sync.dma_start(thw_sb[:, ts(3, 4), :], thw[:, ts(3, 4), :])
            for m in range(4, MN):
                t_ = xh0pool.tile([P, KH, P], BF16, tag="xh0")
                nc.sync.dma_start(t_[:], x_h0[:, m])
                xh0_t.append(t_)
            gw_sb = cpool.tile([P, KL, LOW], BF16, tag="gw")
            nc.sync.dma_start(gw_sb[:], gw[:])
            gbc_sb = cpool.tile([P, KL], BF16, tag="gbc")
            nc.sync.dma_start(gbc_sb[:], gbc[:])
            bnt_sb = cpool.tile([P, KH], F32, tag="bnt")
            nc.sync.dma_start(bnt_sb[:], bnt[:])
            xl_sb[1] = xlpool.tile([P, KL, N], BF16, tag="xl", name="xl1_sb")
            nc.sync.dma_start(xl_sb[1][:], x_l[1][:])
            ww_sb = cpool.tile([P, KL, HIGH], BF16, tag="ww")
            nc.sync.dma_start(ww_sb[:], ww[:])
            xh1_t = []
            for k in range(KH):
                t_ = xh1pool.tile([P, N], BF16, tag="xh1")
                nc.sync.dma_start(t_[:], x_h1[ts(k, P), :])
                xh1_t.append(t_)

            # ---------------- persistent intermediates
            ph_sb = [
                phpool.tile([P, MN, LOW], BF16, tag="ph", name=f"ph{i}_sb")
                for i in range(2)
            ]
            th_sb = midpool.tile([P, MN, LOW], BF16, tag="th")
            att_sb = midpool.tile([P, KL, LOW], BF16, tag="att")
            st_sb = midpool.tile([P, KL, LOW], BF16, tag="st")
            agb_sb = midpool.tile([P, KL], F32, tag="agb")
            y_sb = midpool.tile([P, KL, N], BF16, tag="y")

            # ---------------- phase helpers (emission order == schedule)
            def a1_tile(b, m):
                # phi_xT tile: out [n-part m, d 512]
                ps = pspool.tile([P, 512], F32, tag="ps")
                for k in range(KL):
                    nc.tensor.matmul(
                        ps[:], xl_sb[b][:, k, ts(m, P)], phw_sb[:, k, :],
                        start=(k == 0), stop=(k == KL - 1),
                    )
                nc.vector.tensor_tensor(ph_sb[b][:, m, :], ps[:], phb_sb[:], ADD)

            def a2_half_b0(m, k0, k1, name, evac=None):
                # half-k theta pass; evac (if given) copies to SBUF (only one
                # PSUM operand is legal per DVE op, so halves pair via SBUF)
                ps = pspool.tile([P, 512], F32, tag="ps", name=name)
                for k in range(k0, k1):
                    nc.tensor.matmul(
                        ps[:], xh0_t[m][:, k, :], thw_sb[:, k, :],
                        start=(k == k0), stop=(k == k1 - 1),
                    )
                if evac is not None:
                    nc.scalar.activation(evac[:], ps[:], AF.Copy)
                return ps

            def a2_tile_b0(m):
                # theta_xT tile from m-sliced x_h0: out [n-part m, c 512]
                ps = pspool.tile([P, 512], F32, tag="ps")
                for k in range(KH):
                    nc.tensor.matmul(
                        ps[:], xh0_t[m][:, k, :], thw_sb[:, k, :],
                        start=(k == 0), stop=(k == KH - 1),
                    )
                nc.vector.tensor_tensor(th_sb[:, m, :], ps[:], thb_sb[:], ADD)

            def a2_tile_b1(m):
                ps = pspool.tile([P, 512], F32, tag="ps")
                for k in range(KH):
                    nc.tensor.matmul(
                        ps[:], xh1_t[k][:, ts(m, P)], thw_sb[:, k, :],
                        start=(k == 0), stop=(k == KH - 1),
                    )
                nc.vector.tensor_tensor(th_sb[:, m, :], ps[:], thb_sb[:], ADD)

            def b1_tile(b, md):
                # attT tile: out [d-part md, c 512]
                ps = pspool.tile([P, 512], F32, tag="ps")
                for k in range(MN):
                    nc.tensor.matmul(
                        ps[:], ph_sb[b][:, k, ts(md, P)], th_sb[:, k, :],
                        start=(k == 0), stop=(k == MN - 1),
                    )
                nc.scalar.activation(att_sb[:, md, :], ps[:], AF.Copy)

            def b2_tile(ml):
                # stT tile: out [l-part ml, c 512]
                ps = pspool.tile([P, 512], F32, tag="ps")
                for k in range(KL):
                    nc.tensor.matmul(
                        ps[:], gw_sb[:, k, ts(ml, P)], att_sb[:, k, :],
                        start=(k == 0), stop=(k == KL - 1),
                    )
                nc.scalar.activation(st_sb[:, ml, :], ps[:], AF.Copy)

            def agb_tiles():
                # agb[c] = sum_d attT[d,c]·gb[d]: out [c-part mc, 1]
                for mc in range(KL):
                    ps = pspool.tile([P, 512], F32, tag="ps")
                    for k in range(KL):
                        nc.tensor.matmul(
                            ps[:, 0:1], att_sb[:, k, ts(mc, P)], gbc_sb[:, k : k + 1],
                            start=(k == 0), stop=(k == KL - 1),
                        )
                    nc.scalar.activation(agb_sb[:, mc : mc + 1], ps[:, 0:1], AF.Copy)

            def b3_tile(b, nn, mc):
                # y tile: out [c-part mc, n-slice nn]
                ps = pspool.tile([P, 512], F32, tag="ps")
                for k in range(KL):
                    nc.tensor.matmul(
                        ps[:, :NW], st_sb[:, k, ts(mc, P)], xl_sb[b][:, k, ts(nn, NW)],
                        start=(k == 0), stop=(k == KL - 1),
                    )
                nc.scalar.activation(
                    y_sb[:, mc, ts(nn, NW)], ps[:, :NW], AF.Identity,
                    bias=agb_sb[:, mc : mc + 1],
                )

            def c_tile(b, mo, nn):
                # w_y tile + BN shift + residual, then store
                ps = pspool.tile([P, 512], F32, tag="ps")
                for k in range(KL):
                    nc.tensor.matmul(
                        ps[:, :NW], ww_sb[:, k, ts(mo, P)], y_sb[:, k, ts(nn, NW)],
                        start=(k == 0), stop=(k == KL - 1),
                    )
                stg = stgpool.tile([P, NW], BF16, tag="stg")
                if b == 0:
                    for j in range(NSPLIT):
                        nc.vector.scalar_tensor_tensor(
                            stg[:, ts(j, P)], ps[:, ts(j, P)], bnt_sb[:, mo : mo + 1],
                            xh0_t[nn * NSPLIT + j][:, mo, :], ADD, ADD,
                        )
                else:
                    nc.vector.scalar_tensor_tensor(
                        stg[:], ps[:, :NW], bnt_sb[:, mo : mo + 1],
                        xh1_t[mo][:, ts(nn, NW)], ADD, ADD,
                    )
                nc.sync.dma_start(out[b, ts(mo, P), ts(nn, NW)], stg[:])

            # ---------------- the schedule
            for m in range(MN):
                a1_tile(0, m)                       # A1(b0)
            # A2(b0): first 4 m-tiles split k0-7/k8-15 so theta starts with
            # only half of thw + one x_h0 slice landed; rest run full-k
            tha = [
                midpool.tile([P, 512], BF16, tag="tha", bufs=4, name=f"tha{m}")
                for m in range(4)
            ]
            for m in range(4):
                a2_half_b0(m, 0, KH // 2, f"pa{m}", evac=tha[m])
            for m in range(4):
                pb = a2_half_b0(m, KH // 2, KH, f"pb{m}")
                nc.vector.tensor_tensor(th_sb[:, m, :], pb[:], tha[m][:], ADD)
                nc.vector.tensor_tensor(
                    th_sb[:, m, :], th_sb[:, m, :], thb_sb[:], ADD)
            for m in range(4, MN):
                a2_tile_b0(m)                       # A2(b0), streams x_h0
            a1_tile(1, 0)                           # filler: th evac latency
            a1_tile(1, 1)
            for md in range(KL):
                b1_tile(0, md)                      # B1(b0)
            a1_tile(1, 2)                           # filler: att evac
            for ml in range(KL):
                b2_tile(ml)                         # B2(b0)
            agb_tiles()
            a1_tile(1, 3)                           # filler: st evac
            for nn in range(NSPLIT):
                for mc in range(KL):
                    b3_tile(0, nn, mc)              # B3(b0), nn-outer
            for nn in range(2):
                for mo in range(KH):
                    c_tile(0, mo, nn)               # C(b0) nn0+nn1
            for m in range(4, MN):
                a1_tile(1, m)                       # A1(b1) rest
            for m in range(MN):
                a2_tile_b1(m)                       # A2(b1), x_h1 resident
            for mo in range(0, 4):
                c_tile(0, mo, 2)                    # filler: th(b1) evac
            for md in range(KL):
                b1_tile(1, md)                      # B1(b1)
            for mo in range(4, 8):
                c_tile(0, mo, 2)                    # filler: att evac
            for ml in range(KL):
                b2_tile(ml)                         # B2(b1)
            agb_tiles()
            for mo in range(8, KH):
                c_tile(0, mo, 2)                    # filler: st evac + C(b0) tail
            for nn in range(NSPLIT):
                for mc in range(KL):
                    b3_tile(1, nn, mc)              # B3(b1)
            for nn in range(NSPLIT):
                for mo in range(KH):
                    c_tile(1, mo, nn)               # C(b1)
    nc.compile()
    return nc


_CACHE: dict = {}


def _get_module() -> bass.Bass:
    if "nc" not in _CACHE:
        _CACHE["nc"] = _build_module()
    return _CACHE["nc"]


def _prep_maps(inputs: dict) -> list[dict]:
    import ml_dtypes

    bf = ml_dtypes.bfloat16
    f = lambda a: np.asarray(a, dtype=np.float32)
    x_h = f(inputs["x_h"]).reshape(B, HIGH, N)
    x_l = f(inputs["x_l"]).reshape(B, LOW, N)
    theta_w = f(inputs["theta_w"])
    phi_w = f(inputs["phi_w"])
    g_w = f(inputs["g_w"])
    w_w = f(inputs["w_w"])

    thw_h = np.ascontiguousarray(
        theta_w.T.reshape(KH, P, LOW).transpose(1, 0, 2).astype(bf))
    phw_h = np.ascontiguousarray(
        (phi_w.T / np.float32(LOW)).reshape(KL, P, LOW).transpose(1, 0, 2).astype(bf))
    # gw in [d-part, l] layout (NOT transposed): gw_h[p,k,l] = g_w[k*128+p, l]
    gw_h = np.ascontiguousarray(g_w.reshape(KL, P, LOW).transpose(1, 0, 2).astype(bf))
    s = f(inputs["bn_gamma"]) / np.sqrt(f(inputs["bn_var"]) + np.float32(BN_EPS))
    # BN scale folded into the w conv weights; only the shift remains on-device
    ww_h = np.ascontiguousarray(
        (w_w * s[:, None]).T.reshape(KL, P, HIGH).transpose(1, 0, 2).astype(bf))

    thpb_h = np.concatenate(
        [f(inputs["theta_b"]), f(inputs["phi_b"]) / np.float32(LOW)]
    ).reshape(1, 2 * LOW).astype(bf)
    gbc_h = np.ascontiguousarray(f(inputs["g_b"]).reshape(KL, P).T.astype(bf))
    t = (f(inputs["w_b"]) - f(inputs["bn_mean"])) * s + f(inputs["bn_beta"])
    bnt_h = np.ascontiguousarray(t.astype(np.float32).reshape(KH, P).T)

    x_h_bf = x_h.astype(bf)
    x_l_bf = x_l.astype(bf)
    shared = dict(
        thw=thw_h, phw=phw_h, gw=gw_h, ww=ww_h, thpb=thpb_h, gbc=gbc_h, bnt=bnt_h,
    )
    maps = []
    for c in range(NCORES):
        m = dict(shared)
        b0 = x_h_bf[c * BPC]
        m["x_h0"] = np.ascontiguousarray(
            b0.reshape(KH, P, MN, P).transpose(1, 2, 0, 3))
        m["x_h1"] = np.ascontiguousarray(x_h_bf[c * BPC + 1])
        m["x_l"] = np.ascontiguousarray(
            x_l_bf[c * BPC : (c + 1) * BPC].reshape(BPC, KL, P, N).transpose(0, 2, 1, 3))
        maps.append(m)
    return maps


def _run(inputs: dict, **kwargs):
    from concourse.bass_utils import run_bass_kernel_spmd

    nc = _get_module()
    in_maps = _prep_maps(inputs)
    res = run_bass_kernel_spmd(nc, in_maps, core_ids=list(range(NCORES)), **kwargs)
    parts = [np.asarray(r["out"], dtype=np.float32) for r in res.results]
    full = np.concatenate(parts, axis=0).reshape(B, HIGH, H, W)
    return full, res


def kernel(**inputs) -> np.ndarray:
    full, _ = _run(inputs)
    return full


# revision 39
# speedup vs baseline: 1.1553x; 1.0089x over previous
"""Trainium2 Bass kernel for the non-local-block module (nn_CNL_747324309589).

Sharding: data-parallel over batch — 16 batches across 8 NeuronCores, 2 per
core, no collectives.  Per batch (dims: HIGH=2048, LOW=512, N=H*W=1152):

    theta_xT[n,c] = sum_h xh[h,n]·thwT[h,c] + thb[c]       (A2)
    phi_xT [n,d]  = sum_l xl[l,n]·phwT[l,d] + phb[d]       (A1, phw,phb /512)
    attT   [d,c]  = sum_n phi_xT[n,d]·theta_xT[n,c]        (B1, = energy^T/512)
    stT    [l,c]  = sum_d gw[d,l]·attT[d,c]                (B2, = (att@g_w)^T)
    agb    [c]    = sum_d attT[d,c]·gb[d]                  (AGB, tiny)
    y      [c,n]  = sum_l stT[l,c]·xl[l,n] + agb[c]        (B3)
    w_y    [o,n]  = sum_c wwT[c,o]·y[c,n]                  (C, BN scale in ww)
    out    [o,n]  = w_y + bnt[o] + xh[o,n]                 (stst + residual)

The B2/B3 re-association (y = (att·[g_w])·x_l + (att·g_b)) eliminates the
separate g_x = g_w·x_l + g_b phase: 8192+16 rows replace 18432 per batch.

Everything moves as bfloat16 (PE rate for bf16 == float32r at 1.0 cycle/row,
but DMA bytes and SBUF halve: total DMA ~76us vs PE ~176us so the single
DMA resource never paces the kernel).  PSUM accumulates fp32.  Measured
end-to-end rel err ~6e-3 (gate 2e-2).

Schedule: batch 0's theta streams x_h m-column-slices (host pre-tiled
[p, m, kh, 128]) so the first theta tile starts right after A1; batch 1's
x_h is fully resident by its theta (normal [kh, n] chunks).  Cross-batch
filler tiles (A1 of b1 / held-back C tiles of b0) cover every phase-boundary
PSUM-evacuation latency so the PE never idles mid-kernel.
"""

import numpy as np

import concourse.bass as bass
import concourse.bacc as bacc
import concourse.mybir as mybir
import concourse.tile as tile
from concourse.bass import ts

B, HIGH, LOW, H, W = 16, 2048, 512, 48, 24
N = H * W            # 1152
NCORES = 8
BPC = B // NCORES    # 2 batches per core
P = 128
KH = HIGH // P       # 16
KL = LOW // P        # 4
MN = N // P          # 9
NSPLIT = 3
NW = N // NSPLIT     # 384
BN_EPS = 1e-5

F32 = mybir.dt.float32
BF16 = mybir.dt.bfloat16
ADD = mybir.AluOpType.add
AF = mybir.ActivationFunctionType


def _build_module() -> bass.Bass:
    nc = bacc.Bacc()
    # batch-0 x_h pre-tiled by n-column block m: x_h0[p, m, kh, c] = xh[kh*128+p, m*128+c]
    x_h0 = nc.dram_tensor("x_h0", [P, MN, KH, P], BF16, kind="ExternalInput")
    # batch-1 x_h in plain channel-major layout
    x_h1 = nc.dram_tensor("x_h1", [HIGH, N], BF16, kind="ExternalInput")
    x_l = nc.dram_tensor("x_l", [BPC, P, KL, N], BF16, kind="ExternalInput")
    thw = nc.dram_tensor("thw", [P, KH, LOW], BF16, kind="ExternalInput")
    phw = nc.dram_tensor("phw", [P, KL, LOW], BF16, kind="ExternalInput")
    gw = nc.dram_tensor("gw", [P, KL, LOW], BF16, kind="ExternalInput")
    ww = nc.dram_tensor("ww", [P, KL, HIGH], BF16, kind="ExternalInput")
    thpb = nc.dram_tensor("thpb", [1, 2 * LOW], BF16, kind="ExternalInput")
    gbc = nc.dram_tensor("gbc", [P, KL], BF16, kind="ExternalInput")
    bnt = nc.dram_tensor("bnt", [P, KH], F32, kind="ExternalInput")
    out = nc.dram_tensor("out", [BPC, HIGH, N], BF16, kind="ExternalOutput")

    with tile.TileContext(nc) as tc:
        with (
            tc.tile_pool(name="consts", bufs=1) as cpool,
            tc.tile_pool(name="xh0", bufs=MN) as xh0pool,
            tc.tile_pool(name="xh1", bufs=KH) as xh1pool,
            tc.tile_pool(name="xl", bufs=2) as xlpool,
            tc.tile_pool(name="ph", bufs=2) as phpool,
            tc.tile_pool(name="mid", bufs=1) as midpool,
            tc.tile_pool(name="stg", bufs=8) as stgpool,
            tc.tile_pool(name="psum", bufs=8, space="PSUM") as pspool,
        ):
            # ---------------- PE warmup: dummy matmuls on a memset tile keep
            # the PE busy through the DMA prologue so the p-state ramp (half
            # clock for the first 3us of a busy stretch) is spent on idle
            # time instead of real matmuls.
            wt_sb = cpool.tile([P, 64], BF16, tag="wt")
            nc.vector.memset(wt_sb[:], 0)
            wps = pspool.tile([P, 512], F32, tag="ps", name="wps")
            # pe_busy_start is pinned by the FIRST PE instruction and never
            # resets, so dummies started at ~0.4us make every real matmul run
            # at full clock.  They also keep the next real matmul OFF the
            # queue head until its data has landed: an instruction that
            # blocks on a DMA sem wakes ~1.7us after the transfer, while one
            # arriving after the data is visible proceeds within ~50ns.
            for _ in range(50):
                nc.tensor.matmul(wps[0:64, 0:64], wt_sb[:], wt_sb[:],
                                 start=True, stop=True)

            def pad(n):
                # dummy matmuls that absorb a known DMA-arrival slack so the
                # following real matmul never blocks (see above)
                pps = pspool.tile([P, 512], F32, tag="ps", name="padps")
                for _ in range(n):
                    nc.tensor.matmul(pps[0:64, 0:64], wt_sb[:], wt_sb[:],
                                     start=True, stop=True)

            # ---------------- DMA stream (single global DMA resource; this
            # order is the delivery schedule the PE schedule below relies on)
            phw_sb = cpool.tile([P, KL, LOW], BF16, tag="phw")
            nc.sync.dma_start(phw_sb[:], phw[:])
            xl_sb = [None, None]
            xl_sb[0] = xlpool.tile([P, KL, N], BF16, tag="xl", name="xl0_sb")
            nc.sync.dma_start(xl_sb[0][:, :, 0:NW], x_l[0][:, :, 0:NW])
            nc.sync.dma_start(xl_sb[0][:, :, NW:N], x_l[0][:, :, NW:N])
            # thw quarters in separate tiles: consumers of quarter i must not
            # serialize on later quarters' DMAs
            thwq = []
            xh0_t = []
            for q in range(2):
                t_ = cpool.tile([P, 4, LOW], BF16, tag=f"thwq{q}", name=f"thwq{q}")
                nc.sync.dma_start(t_[:], thw[:, ts(q, 4), :])
                thwq.append(t_)
            for m in range(3):
                t_ = xh0pool.tile([P, KH, P], BF16, tag="xh0")
                nc.sync.dma_start(t_[:], x_h0[:, m])
                xh0_t.append(t_)
            for q in range(2, 4):
                t_ = cpool.tile([P, 4, LOW], BF16, tag=f"thwq{q}", name=f"thwq{q}")
                nc.sync.dma_start(t_[:], thw[:, ts(q, 4), :])
                thwq.append(t_)
            # biases land after the theta-critical stream; batch-0 A1 evacs
            # are raw Act copies with the phi bias added in a deferred fixup
            thpb_sb = cpool.tile([P, 2 * LOW], BF16, tag="thpb")
            nc.sync.dma_start(thpb_sb[:], thpb[:].to_broadcast((P, 2 * LOW)))
            thb_sb = thpb_sb[:, :LOW]
            phb_sb = thpb_sb[:, LOW:]
            for m in range(3, MN):
                t_ = xh0pool.tile([P, KH, P], BF16, tag="xh0")
                nc.sync.dma_start(t_[:], x_h0[:, m])
                xh0_t.append(t_)
            gw_sb = cpool.tile([P, KL, LOW], BF16, tag="gw")
            nc.sync.dma_start(gw_sb[:], gw[:])
            gbc_sb = cpool.tile([P, KL], BF16, tag="gbc")
            nc.sync.dma_start(gbc_sb[:], gbc[:])
            bnt_sb = cpool.tile([P, KH], F32, tag="bnt")
            nc.sync.dma_start(bnt_sb[:], bnt[:])
            xl_sb[1] = xlpool.tile([P, KL, N], BF16, tag="xl", name="xl1_sb")
            nc.sync.dma_start(xl_sb[1][:], x_l[1][:])
            ww_sb = cpool.tile([P, KL, HIGH], BF16, tag="ww")
            nc.sync.dma_start(ww_sb[:], ww[:])
            xh1_t = []
            for k in range(KH):
                t_ = xh1pool.tile([P, N], BF16, tag="xh1")
                nc.sync.dma_start(t_[:], x_h1[ts(k, P), :])
                xh1_t.append(t_)

            # ---------------- persistent intermediates
            ph_sb = [
                phpool.tile([P, MN, LOW], BF16, tag="ph", name=f"ph{i}_sb")
                for i in range(2)
            ]
            th_sb = midpool.tile([P, MN, LOW], BF16, tag="th")
            att_sb = midpool.tile([P, KL, LOW], BF16, tag="att")
            st_sb = midpool.tile([P, KL, LOW], BF16, tag="st")
            agb_sb = midpool.tile([P, KL], F32, tag="agb")
            y_sb = midpool.tile([P, KL, N], BF16, tag="y")

            # ---------------- phase helpers (emission order == schedule)
            def a1_tile(b, m):
                # phi_xT tile: out [n-part m, d 512].  Batch 0 evacuates raw
                # (bias comes via the deferred fixup, so the prologue never
                # waits for the bias DMA); batch 1 fuses the bias add.
                ps = pspool.tile([P, 512], F32, tag="ps")
                for k in range(KL):
                    nc.tensor.matmul(
                        ps[:], xl_sb[b][:, k, ts(m, P)], phw_sb[:, k, :],
                        start=(k == 0), stop=(k == KL - 1),
                    )
                if b == 0:
                    nc.scalar.activation(ph_sb[b][:, m, :], ps[:], AF.Copy)
                else:
                    nc.vector.tensor_tensor(ph_sb[b][:, m, :], ps[:], phb_sb[:], ADD)

            def a2_half_b0(m, k0, k1, name, evac=None):
                # half-k theta pass; evac (if given) copies to SBUF (only one
                # PSUM operand is legal per DVE op, so halves pair via SBUF)
                ps = pspool.tile([P, 512], F32, tag="ps", name=name)
                for k in range(k0, k1):
                    nc.tensor.matmul(
                        ps[:], xh0_t[m][:, k, :], thwq[k // 4][:, k % 4, :],
                        start=(k == k0), stop=(k == k1 - 1),
                    )
                if evac is not None:
                    nc.scalar.activation(evac[:], ps[:], AF.Copy)
                return ps

            def a2_tile_b0(m):
                # theta_xT tile from m-sliced x_h0: out [n-part m, c 512]
                ps = pspool.tile([P, 512], F32, tag="ps")
                for k in range(KH):
                    nc.tensor.matmul(
                        ps[:], xh0_t[m][:, k, :], thwq[k // 4][:, k % 4, :],
                        start=(k == 0), stop=(k == KH - 1),
                    )
                nc.vector.tensor_tensor(th_sb[:, m, :], ps[:], thb_sb[:], ADD)

            def a2_tile_b1(m):
                ps = pspool.tile([P, 512], F32, tag="ps")
                for k in range(KH):
                    nc.tensor.matmul(
                        ps[:], xh1_t[k][:, ts(m, P)], thwq[k // 4][:, k % 4, :],
                        start=(k == 0), stop=(k == KH - 1),
                    )
                nc.vector.tensor_tensor(th_sb[:, m, :], ps[:], thb_sb[:], ADD)

            def b1_tile(b, md):
                # attT tile: out [d-part md, c 512]
                ps = pspool.tile([P, 512], F32, tag="ps")
                for k in range(MN):
                    nc.tensor.matmul(
                        ps[:], ph_sb[b][:, k, ts(md, P)], th_sb[:, k, :],
                        start=(k == 0), stop=(k == MN - 1),
                    )
                nc.scalar.activation(att_sb[:, md, :], ps[:], AF.Copy)

            def b2_tile(ml):
                # stT tile: out [l-part ml, c 512]
                ps = pspool.tile([P, 512], F32, tag="ps")
                for k in range(KL):
                    nc.tensor.matmul(
                        ps[:], gw_sb[:, k, ts(ml, P)], att_sb[:, k, :],
                        start=(k == 0), stop=(k == KL - 1),
                    )
                nc.scalar.activation(st_sb[:, ml, :], ps[:], AF.Copy)

            def agb_tiles():
                # agb[c] = sum_d attT[d,c]·gb[d]: out [c-part mc, 1]
                for mc in range(KL):
                    ps = pspool.tile([P, 512], F32, tag="ps")
                    for k in range(KL):
                        nc.tensor.matmul(
                            ps[:, 0:1], att_sb[:, k, ts(mc, P)], gbc_sb[:, k : k + 1],
                            start=(k == 0), stop=(k == KL - 1),
                        )
                    nc.scalar.activation(agb_sb[:, mc : mc + 1], ps[:, 0:1], AF.Copy)

            def b3_tile(b, nn, mc):
                # y tile: out [c-part mc, n-slice nn]
                ps = pspool.tile([P, 512], F32, tag="ps")
                for k in range(KL):
                    nc.tensor.matmul(
                        ps[:, :NW], st_sb[:, k, ts(mc, P)], xl_sb[b][:, k, ts(nn, NW)],
                        start=(k == 0), stop=(k == KL - 1),
                    )
                nc.scalar.activation(
                    y_sb[:, mc, ts(nn, NW)], ps[:, :NW], AF.Identity,
                    bias=agb_sb[:, mc : mc + 1],
                )

            def c_evac(stg_ap, ps_ap, mo, xh_ap, pair):
                # psum -> stg with BN shift + residual.  Pool cannot read
                # PSUM, so the off-DVE variant goes Act (psum+bias, psum is
                # legal there) then Pool (SBUF-only residual add).
                if pair:
                    t = stgpool.tile(list(stg_ap.shape), BF16, tag="cq", name="cq")
                    nc.scalar.activation(
                        t[:], ps_ap, AF.Identity, bias=bnt_sb[:, mo : mo + 1])
                    nc.gpsimd.tensor_tensor(stg_ap, t[:], xh_ap, ADD)
                else:
                    nc.vector.scalar_tensor_tensor(
                        stg_ap, ps_ap, bnt_sb[:, mo : mo + 1], xh_ap, ADD, ADD)

            def c_tile(b, mo, nn):
                # w_y tile + BN shift + residual, then store.  Evacs and
                # stores alternate across engines/queues so no single queue
                # develops a backlog that drags the kernel tail.
                ps = pspool.tile([P, 512], F32, tag="ps")
                for k in range(KL):
                    nc.tensor.matmul(
                        ps[:, :NW], ww_sb[:, k, ts(mo, P)], y_sb[:, k, ts(nn, NW)],
                        start=(k == 0), stop=(k == KL - 1),
                    )
                stg = stgpool.tile([P, NW], BF16, tag="stg")
                pair = mo % 2 == 1
                if b == 0:
                    for j in range(NSPLIT):
                        c_evac(stg[:, ts(j, P)], ps[:, ts(j, P)], mo,
                               xh0_t[nn * NSPLIT + j][:, mo, :], pair)
                else:
                    c_evac(stg[:], ps[:, :NW], mo, xh1_t[mo][:, ts(nn, NW)], pair)
                deng = nc.scalar if mo % 2 == 0 else nc.sync
                deng.dma_start(out[b, ts(mo, P), ts(nn, NW)], stg[:])

            def c_tile_last(b, mo, nn):
                # final tile: per-128-column psum groups so the very last
                # matmul -> evac -> store chain is a third of the width
                for j in range(NSPLIT):
                    ps = pspool.tile([P, 512], F32, tag="ps")
                    for k in range(KL):
                        nc.tensor.matmul(
                            ps[:, :P],
                            ww_sb[:, k, ts(mo, P)],
                            y_sb[:, k, nn * NW + j * P : nn * NW + (j + 1) * P],
                            start=(k == 0), stop=(k == KL - 1),
                        )
                    stg = stgpool.tile([P, P], BF16, tag="stgl")
                    c_evac(stg[:], ps[:, :P], mo,
                           xh1_t[mo][:, nn * NW + j * P : nn * NW + (j + 1) * P],
                           j != 1)
                    deng = [nc.scalar, nc.sync, nc.gpsimd][j]
                    deng.dma_start(
                        out[b, ts(mo, P), nn * NW + j * P : nn * NW + (j + 1) * P],
                        stg[:],
                    )

            # ---------------- the schedule
            for m in range(MN):
                a1_tile(0, m)                       # A1(b0)
            # A2(b0): first 4 m-tiles split k0-7/k8-15, interleaved with the
            # DMA stream order (x_h0 slices and thw quarters alternate) so
            # theta starts right at A1-end with only q0/q1 + slice 0 landed
            tha = [
                midpool.tile([P, 512], BF16, tag="tha", bufs=4, name=f"tha{m}")
                for m in range(4)
            ]

            def a2b_tile(m):
                pb = a2_half_b0(m, KH // 2, KH, f"pb{m}")
                nc.vector.tensor_tensor(th_sb[:, m, :], pb[:], tha[m][:], ADD)
                nc.vector.tensor_tensor(
                    th_sb[:, m, :], th_sb[:, m, :], thb_sb[:], ADD)

            for m in range(3):
                a2_half_b0(m, 0, KH // 2, f"pa{m}", evac=tha[m])
            for m in range(3):
                a2b_tile(m)
            a2_half_b0(3, 0, KH // 2, "pa3", evac=tha[3])
            a2b_tile(3)
            for m in range(4, MN):
                a2_tile_b0(m)                       # A2(b0), streams x_h0
            for m in range(MN):
                nc.vector.tensor_tensor(           # deferred phi-bias fixup
                    ph_sb[0][:, m, :], ph_sb[0][:, m, :], phb_sb[:], ADD)
            a1_tile(1, 0)                           # filler: th evac latency
            a1_tile(1, 1)
            for md in range(KL):
                b1_tile(0, md)                      # B1(b0)
            a1_tile(1, 2)                           # filler: att evac
            for ml in range(KL):
                b2_tile(ml)                         # B2(b0)
            agb_tiles()
            a1_tile(1, 3)                           # filler: st evac
            for nn in range(NSPLIT):
                for mc in range(KL):
                    b3_tile(0, nn, mc)              # B3(b0), nn-outer
            for nn in range(2):
                for mo in range(KH):
                    c_tile(0, mo, nn)               # C(b0) nn0+nn1
            for m in range(4, MN):
                a1_tile(1, m)                       # A1(b1) rest
            for m in range(MN):
                a2_tile_b1(m)                       # A2(b1), x_h1 resident
            for mo in range(0, 4):
                c_tile(0, mo, 2)                    # filler: th(b1) evac
            for md in range(KL):
                b1_tile(1, md)                      # B1(b1)
            for mo in range(4, 8):
                c_tile(0, mo, 2)                    # filler: att evac
            for ml in range(KL):
                b2_tile(ml)                         # B2(b1)
            agb_tiles()
            for mo in range(8, KH):
                c_tile(0, mo, 2)                    # filler: st evac + C(b0) tail
            for nn in range(NSPLIT):
                for mc in range(KL):
                    b3_tile(1, nn, mc)              # B3(b1)
            for nn in range(NSPLIT):
                for mo in range(KH):
                    if nn == NSPLIT - 1 and mo == KH - 1:
                        c_tile_last(1, mo, nn)      # short final store chain
                    else:
                        c_tile(1, mo, nn)           # C(b1)
    nc.compile()
    return nc


_CACHE: dict = {}


def _get_module() -> bass.Bass:
    if "nc" not in _CACHE:
        _CACHE["nc"] = _build_module()
    return _CACHE["nc"]


def _prep_maps(inputs: dict) -> list[dict]:
    import ml_dtypes

    bf = ml_dtypes.bfloat16
    f = lambda a: np.asarray(a, dtype=np.float32)
    x_h = f(inputs["x_h"]).reshape(B, HIGH, N)
    x_l = f(inputs["x_l"]).reshape(B, LOW, N)
    theta_w = f(inputs["theta_w"])
    phi_w = f(inputs["phi_w"])
    g_w = f(inputs["g_w"])
    w_w = f(inputs["w_w"])

    thw_h = np.ascontiguousarray(
        theta_w.T.reshape(KH, P, LOW).transpose(1, 0, 2).astype(bf))
    phw_h = np.ascontiguousarray(
        (phi_w.T / np.float32(LOW)).reshape(KL, P, LOW).transpose(1, 0, 2).astype(bf))
    # gw in [d-part, l] layout (NOT transposed): gw_h[p,k,l] = g_w[k*128+p, l]
    gw_h = np.ascontiguousarray(g_w.reshape(KL, P, LOW).transpose(1, 0, 2).astype(bf))
    s = f(inputs["bn_gamma"]) / np.sqrt(f(inputs["bn_var"]) + np.float32(BN_EPS))
    # BN scale folded into the w conv weights; only the shift remains on-device
    ww_h = np.ascontiguousarray(
        (w_w * s[:, None]).T.reshape(KL, P, HIGH).transpose(1, 0, 2).astype(bf))

    thpb_h = np.concatenate(
        [f(inputs["theta_b"]), f(inputs["phi_b"]) / np.float32(LOW)]
    ).reshape(1, 2 * LOW).astype(bf)
    gbc_h = np.ascontiguousarray(f(inputs["g_b"]).reshape(KL, P).T.astype(bf))
    t = (f(inputs["w_b"]) - f(inputs["bn_mean"])) * s + f(inputs["bn_beta"])
    bnt_h = np.ascontiguousarray(t.astype(np.float32).reshape(KH, P).T)

    x_h_bf = x_h.astype(bf)
    x_l_bf = x_l.astype(bf)
    shared = dict(
        thw=thw_h, phw=phw_h, gw=gw_h, ww=ww_h, thpb=thpb_h, gbc=gbc_h, bnt=bnt_h,
    )
    maps = []
    for c in range(NCORES):
        m = dict(shared)
        b0 = x_h_bf[c * BPC]
        m["x_h0"] = np.ascontiguousarray(
            b0.reshape(KH, P, MN, P).transpose(1, 2, 0, 3))
        m["x_h1"] = np.ascontiguousarray(x_h_bf[c * BPC + 1])
        m["x_l"] = np.ascontiguousarray(
            x_l_bf[c * BPC : (c + 1) * BPC].reshape(BPC, KL, P, N).transpose(0, 2, 1, 3))
        maps.append(m)
    return maps


def _run(inputs: dict, **kwargs):
    from concourse.bass_utils import run_bass_kernel_spmd

    nc = _get_module()
    in_maps = _prep_maps(inputs)
    res = run_bass_kernel_spmd(nc, in_maps, core_ids=list(range(NCORES)), **kwargs)
    parts = [np.asarray(r["out"], dtype=np.float32) for r in res.results]
    full = np.concatenate(parts, axis=0).reshape(B, HIGH, H, W)
    return full, res


def kernel(**inputs) -> np.ndarray:
    full, _ = _run(inputs)
    return full
